# revision 1
# baseline (speedup 1.0000x reference)
"""Trainium2 Bass kernel for nn_EquivariantHardAlignmentModel.

8 NeuronCores, SPMD (identical program, per-core data):
  - The three 512-step LSTM recurrences (enc fwd, enc bwd, dec) are
    replicated on every core with the full batch of 32: per-step PE cost is
    set by streaming Whh^T and is batch-independent, so replication avoids
    all cross-core traffic.  Phase A = fwd + first half of bwd, phase B =
    dec (as two 16-batch chains) + second half of bwd, so >=2 independent
    chains always pipeline the engines.
  - The G-stack (embed/conv/logits/Z), ys gathers, bilinear alignment and
    loss tail are data-parallel: each core does 4 of 32 batch rows.  Inputs
    are batch-permuted per core so its rows are always rows 0..3 -> one
    shared program.
  - p[b,j] = log(sum_i exp(lys+eij-lnZ)) - log(sum_i exp(eij)) via
    PSUM-accumulated matmuls + ACT Exp(accum_out).  Host sums & negates.
"""

import os
import sys

sys.path.insert(0, "/opt/trn_rl_repo")

import numpy as np
import ml_dtypes

import concourse.bass as bass
import concourse.mybir as mybir
import concourse.tile as tile
from concourse import bacc
from concourse.bass_utils import run_bass_kernel_spmd
from concourse.masks import make_identity

BF = mybir.dt.bfloat16
F32 = mybir.dt.float32
AF = mybir.ActivationFunctionType

B, NE, ND = 32, 512, 512
V = 2000
H, F, KW, PG = 256, 256, 5, 4
EE, ED = 128, 128
NCORES, BPC = 8, 4

# torch gate blocks i,f,g,o (256 each) -> reordered [i f o g]
GATE_PERM = np.concatenate(
    [np.arange(0, 512), np.arange(768, 1024), np.arange(512, 768)]
)


def _bf(x):
    return np.ascontiguousarray(x.astype(ml_dtypes.bfloat16))


def _wrap16(flat):
    """index list -> (128, n/16) int16, dma_gather wrapped + 8x replicated."""
    flat = np.asarray(flat).reshape(-1)
    assert flat.size % 16 == 0
    w = flat.reshape(-1, 16).T.astype(np.int16)  # (16, n/16)
    return np.ascontiguousarray(np.tile(w, (8, 1)))


# ---------------------------------------------------------------------------
# device program
# ---------------------------------------------------------------------------

def build_program(n_enc=NE, n_dec=ND):
    from contextlib import ExitStack

    nc = bacc.Bacc(None, target_bir_lowering=False, debug=False)

    with tile.TileContext(nc) as tc, ExitStack() as es:
        dram = es.enter_context(tc.tile_pool(name="dram", bufs=1, space="DRAM"))

        def din(name, shape, dtype):
            return dram.tile(shape, dtype, kind="ExternalInput", name=name,
                             uniquify=False)

        x_enc_idx = din("x_enc_idx", [128, B * n_enc // 16], mybir.dt.int16)
        y_dec_idx = din("y_dec_idx", [128, B * n_dec // 16], mybir.dt.int16)
        e_idx = din("e_idx", [128, BPC * NE // 16], mybir.dt.int16)
        gb_idx = din("gb_idx", [128, BPC * ND // 16], mybir.dt.int16)
        gembed_bf = din("gembed_bf", [V, F], BF)
        enc_embed_bf = din("enc_embed_bf", [V, EE], BF)
        dec_embed_bf = din("dec_embed_bf", [V, ED], BF)
        w2t_bf = din("w2t_bf", [V, F], BF)
        w2_d = din("w2_d", [128, 2, V], BF)
        gconv_d = din("gconv_d", [128, KW * 4, 128], BF)
        wih_d = din("wih_d", [128, 4 * H], BF)
        whh_d = din("whh_d", [128, 2, 4 * H], BF)
        wiy_d = din("wiy_d", [128, 4 * H], BF)
        whd_d = din("whd_d", [128, 2, 4 * H], BF)
        tt_d = din("tt_d", [128, 8, 128], BF)
        pout = dram.tile([128, 16], F32, kind="ExternalOutput", name="pout",
                         uniquify=False)

        cpool = es.enter_context(tc.tile_pool(name="const", bufs=1))

        idbf = cpool.tile([128, 128], BF)
        make_identity(nc, idbf[:])
        idf32 = cpool.tile([128, 128], F32)
        make_identity(nc, idf32[:])
        negones = cpool.tile([1, 128], F32)
        nc.gpsimd.memset(negones[:], -1.0)

        def to_sbuf(ap, name):
            t = cpool.tile(list(ap.shape), ap.dtype, name=name)
            nc.sync.dma_start(out=t[:], in_=ap[:])
            return t

        w2_sb = to_sbuf(w2_d, "w2_sb")
        gconv_sb = to_sbuf(gconv_d, "gconv_sb")
        wih_sb = to_sbuf(wih_d, "wih_sb")
        whh_sb = to_sbuf(whh_d, "whh_sb")
        wiy_sb = to_sbuf(wiy_d, "wiy_sb")
        whd_sb = to_sbuf(whd_d, "whd_sb")
        tt_sb = to_sbuf(tt_d, "tt_sb")
        xidx_sb = to_sbuf(x_enc_idx, "xidx_sb")
        yidx_sb = to_sbuf(y_dec_idx, "yidx_sb")
        eidx_sb = to_sbuf(e_idx, "eidx_sb")
        gbidx_sb = to_sbuf(gb_idx, "gbidx_sb")

        gpool = es.enter_context(tc.tile_pool(name="gath", bufs=1))
        GCH = 4096

        def big_gather(out_t, table, idx_sb, n):
            for k0 in range(0, n, GCH):
                c = min(GCH, n - k0)
                nc.gpsimd.dma_gather(
                    out_ap=out_t[:, :, k0:k0 + c], in_ap=table[:],
                    idxs_ap=idx_sb[:, k0 // 16:(k0 + c) // 16],
                    num_idxs=c, num_idxs_reg=c, elem_size=EE,
                    transpose=True, single_packet=False)

        xgath = gpool.tile([128, 1, B * n_enc], BF)
        big_gather(xgath, enc_embed_bf, xidx_sb, B * n_enc)
        ygath = gpool.tile([128, 1, B * n_dec], BF)
        big_gather(ygath, dec_embed_bf, yidx_sb, B * n_dec)

        eT = [gpool.tile([128, 2, NE], BF, name=f"eT{b}") for b in range(BPC)]
        gbT = [gpool.tile([128, 2, ND], BF, name=f"gbT{b}") for b in range(BPC)]

        # persistent activation stores
        spool = es.enter_context(tc.tile_pool(name="stores", bufs=1))
        tcT = [spool.tile([128, 2, NE], BF, name=f"tcT{b}") for b in range(BPC)]
        lnZ = [spool.tile([1, NE], F32, name=f"lnZ{b}") for b in range(BPC)]
        hencTf = spool.tile([128, 2, BPC * NE], BF)
        hencTb = spool.tile([128, 2, BPC * NE], BF)
        hdecT = spool.tile([128, 2, BPC * (ND + 1)], BF)
        pout_sb = spool.tile([128, 16], F32)
        nc.gpsimd.memset(hencTf[:], 0.0)
        nc.gpsimd.memset(hencTb[:], 0.0)
        nc.gpsimd.memset(hdecT[:], 0.0)

        # ------------------------------------------------------------------
        # Phase G
        # ------------------------------------------------------------------
        with tc.tile_pool(name="gwork", bufs=2) as gw, \
             tc.tile_pool(name="gpsum", bufs=2, space="PSUM") as gp, \
             tc.tile_pool(name="zrow", bufs=4, space="PSUM") as zrp:
            for b in range(BPC):
                gsc = gw.tile([128, 2, NE], BF, tag="gathsc")
                nc.gpsimd.dma_gather(
                    out_ap=gsc[:], in_ap=gembed_bf[:],
                    idxs_ap=eidx_sb[:, b * NE // 16:(b + 1) * NE // 16],
                    num_idxs=NE, num_idxs_reg=NE, elem_size=F, transpose=True)
                nc.scalar.activation(eT[b][:], gsc[:], AF.Tanh)
                nc.gpsimd.dma_gather(
                    out_ap=gbT[b][:], in_ap=w2t_bf[:],
                    idxs_ap=gbidx_sb[:, b * ND // 16:(b + 1) * ND // 16],
                    num_idxs=ND, num_idxs_reg=ND, elem_size=F, transpose=True)
            # conv + tanh
            for b in range(BPC):
                for fo in range(2):
                    cp = gp.tile([128, NE], F32, tag="convps")
                    first = True
                    for k in [2, 0, 1, 3, 4]:
                        d = k - 2
                        lo_out, lo_in = max(0, -d), max(0, d)
                        L = NE - abs(d)
                        for fi in range(2):
                            nc.tensor.matmul(
                                cp[:, lo_out:lo_out + L],
                                gconv_sb[:, (k * 2 + fi) * 2 + fo, :],
                                eT[b][:, fi, lo_in:lo_in + L],
                                start=first, stop=(k == 4 and fi == 1),
                                skip_group_check=True)
                            first = False
                    nc.scalar.activation(tcT[b][:, fo, :], cp[:], AF.Tanh)
            # logits (t-major) -> exp -> Z
            zrows = []
            for b in range(BPC):
                zrow = zrp.tile([1, NE], F32, tag="zrow", name=f"zr{b}")
                for ic in range(4):
                    zp = gw.tile([128, 4], F32, tag="zp")
                    for vc in range(4):
                        lp = gp.tile([128, 500], F32, tag="logps")
                        for f in range(2):
                            nc.tensor.matmul(
                                lp[:], tcT[b][:, f, ic * 128:(ic + 1) * 128],
                                w2_sb[:, f, vc * 500:(vc + 1) * 500],
                                start=(f == 0), stop=(f == 1))
                        sc = gw.tile([128, 500], BF, tag="expsc")
                        nc.scalar.activation(sc[:], lp[:], AF.Exp,
                                             accum_out=zp[:, vc:vc + 1])
                    zc = gw.tile([128, 1], F32, tag="zc")
                    nc.vector.tensor_reduce(zc[:], zp[:],
                                            axis=mybir.AxisListType.X,
                                            op=mybir.AluOpType.add)
                    nc.tensor.transpose(zrow[:, ic * 128:(ic + 1) * 128],
                                        zc[:], idf32[:])
                zrows.append(zrow)
            for b in range(BPC):
                nc.scalar.activation(lnZ[b][:], zrows[b][:], AF.Ln)

        # ------------------------------------------------------------------
        # LSTM phases
        # ------------------------------------------------------------------
        lstm_es = ExitStack()
        lst_sb = lstm_es.enter_context(tc.tile_pool(name="lstm_sb", bufs=2))
        lst_ps = lstm_es.enter_context(tc.tile_pool(name="lstm_ps", bufs=3,
                                                    space="PSUM"))
        lst_tp = lstm_es.enter_context(tc.tile_pool(name="lstm_tp", bufs=2,
                                                    space="PSUM"))

        class Chain:
            def __init__(self, name, Bc, xg, wih, whh, store, col_of,
                         xcol, n_steps):
                self.name, self.B = name, Bc
                self.xg, self.wih, self.whh = xg, wih, whh
                self.store, self.col_of = store, col_of
                self.xcol, self.n_steps = xcol, n_steps
                self.hT = None
                self.W = None  # (Bc, 1280) bf16 = [si sf g~ c so]
                self.z_next = None

            def init_zero(self):
                hT = lst_sb.tile([128, 2, self.B], BF, tag=self.name + "hT")
                W = lst_sb.tile([self.B, 1280], BF, tag=self.name + "W",
                                bufs=1, name="W")
                nc.gpsimd.memset(hT[:], 0.0)
                nc.gpsimd.memset(W[:], 0.0)
                self.hT, self.W = hT, W

            def prime(self, t):
                """Issue the x-part matmuls for step t into a fresh z tile."""
                Bc = self.B
                z = lst_ps.tile([B, 1024], F32, tag="xz", bufs=3,
                                name="z")[0:Bc, :]
                xc = self.xg[:, 0, self.xcol(t):self.xcol(t) + Bc]
                nc.tensor.matmul(z[:, 0:512], xc, self.wih[:, 0:512],
                                 start=True, stop=False,
                                 skip_group_check=True)
                nc.tensor.matmul(z[:, 512:1024], xc, self.wih[:, 512:1024],
                                 start=True, stop=False,
                                 skip_group_check=True)
                self.z_next = z

            def s1(self, t):
                Bc = self.B
                z = self.z_next
                # finish z[0:512] first so sigmoid_if can start early
                for nh in range(2):
                    for hc in range(2):
                        nc.tensor.matmul(z[:, nh * 512:nh * 512 + 512],
                                         self.hT[:, hc, :],
                                         self.whh[:, hc,
                                                  nh * 512:nh * 512 + 512],
                                         start=False, stop=(hc == 1),
                                         skip_group_check=True)
                # W = [si sf | g~ | c | so]
                W = self.W
                nc.scalar.activation(W[:, 0:512], z[:, 0:512], AF.Sigmoid)
                nc.scalar.activation(W[:, 512:768], z[:, 512:768], AF.Tanh)

            def s2(self, t):
                Bc, nm = self.B, self.name
                z, W = self.z_next, self.W
                # m1 = sf*c (only needs sigmoid_if); m0 = si*g~; c' = m0+m1
                m = lst_sb.tile([B, 512], BF, tag="xt2", bufs=3,
                                name="m")[0:Bc, :]
                nc.vector.tensor_mul(m[:, 256:512], W[:, 256:512],
                                     W[:, 768:1024])
                nc.vector.tensor_mul(m[:, 0:256], W[:, 0:256],
                                     W[:, 512:768])
                nc.vector.tensor_add(W[:, 768:1024], m[:, 0:256],
                                     m[:, 256:512])
                # so (off critical path)
                nc.scalar.activation(W[:, 1024:1280], z[:, 768:1024],
                                     AF.Sigmoid)
                # pre-issue next step's x matmuls so they don't queue
                # behind the transposes in PE program order
                if t + 1 < self.n_steps:
                    self.prime(t + 1)
                # transpose c' and so; tanh + h-mul done H-major
                psT = lst_tp.tile([128, 4, B], BF, tag="xpsT", bufs=2,
                                  name="psT")[:, :, 0:Bc]
                nc.tensor.transpose(psT[:, 0, :], W[:, 768:896],
                                    idbf[0:Bc, 0:Bc])
                nc.tensor.transpose(psT[:, 1, :], W[:, 896:1024],
                                    idbf[0:Bc, 0:Bc])
                nc.tensor.transpose(psT[:, 2, :], W[:, 1024:1152],
                                    idbf[0:Bc, 0:Bc])
                nc.tensor.transpose(psT[:, 3, :], W[:, 1152:1280],
                                    idbf[0:Bc, 0:Bc])
                thT = lst_sb.tile([128, 2, B], BF, tag="xthT", bufs=3,
                                  name="thT")[:, :, 0:Bc]
                nc.scalar.activation(thT[:], psT[:, 0:2, :], AF.Tanh)
                hT = lst_sb.tile([128, 2, Bc], BF, tag=nm + "hT")
                nc.vector.tensor_mul(hT[:], psT[:, 2:4, :], thT[:])
                if self.store is not None:
                    col = self.col_of(t)
                    stride = self.store.shape[2] // BPC
                    nc.vector.tensor_copy(
                        self.store[:, :, col::stride][:, :, 0:BPC],
                        hT[:, :, 0:BPC])
                self.hT = hT

        fwd = Chain("f", B, xgath, wih_sb, whh_sb, hencTf, lambda t: t,
                    lambda t: t * B, n_enc)
        bwd = Chain("b", B, xgath, wih_sb, whh_sb, hencTb,
                    lambda t: n_enc - 1 - t,
                    lambda t: (n_enc - 1 - t) * B, n_enc)
        fwd.init_zero()
        bwd.init_zero()
        fwd.prime(0)
        bwd.prime(0)

        bt = 0
        for t in range(n_enc):
            fwd.s1(t)
            do_b = (t % 2 == 1 and bt < n_enc // 2)
            if do_b:
                bwd.s1(bt)
            fwd.s2(t)
            if do_b:
                bwd.s2(bt)
                bt += 1

        hTf, Wf = fwd.hT, fwd.W
        nc.vector.tensor_copy(hdecT[:, :, 0::ND + 1][:, :, 0:BPC],
                              hTf[:, :, 0:BPC])

        dec = Chain("d", B, ygath, wiy_sb, whd_sb, hdecT, lambda t: t + 1,
                    lambda t: t * B, n_dec)
        dec.hT, dec.W = hTf, Wf
        dec.prime(0)

        for t in range(n_dec):
            dec.s1(t)
            do_b = (t % 2 == 1 and bt < n_enc)
            if do_b:
                bwd.s1(bt)
            dec.s2(t)
            if do_b:
                bwd.s2(bt)
                bt += 1
        while bt < n_enc:
            bwd.s1(bt)
            bwd.s2(bt)
            bt += 1

        lstm_es.close()

        # ------------------------------------------------------------------
        # Final phase
        # ------------------------------------------------------------------
        with tc.tile_pool(name="fin_sb", bufs=2) as fsb, \
             tc.tile_pool(name="fin_keep", bufs=1) as fkeep, \
             tc.tile_pool(name="fin_ps", bufs=2, space="PSUM") as fps:
            sda = [fkeep.tile([128, 8], F32, name=f"sda{b}")
                   for b in range(BPC)]
            for b in range(BPC):
                thT = fsb.tile([128, 2, NE], BF, tag="thT")
                for hc in range(2):
                    tp = fps.tile([128, NE], F32, tag="thps")
                    for ec in range(4):
                        src = hencTf if ec < 2 else hencTb
                        nc.tensor.matmul(
                            tp[:], tt_sb[:, ec * 2 + hc, :],
                            src[:, ec % 2, b * NE:(b + 1) * NE],
                            start=(ec == 0), stop=(ec == 3))
                    nc.scalar.activation(thT[:, hc, :], tp[:], AF.Copy)
                for jc in range(4):
                    fp = fps.tile([128, NE], F32, tag="fps")
                    for hc in range(2):
                        nc.tensor.matmul(
                            fp[:],
                            hdecT[:, hc, :][:, b * (ND + 1) + jc * 128:
                                            b * (ND + 1) + jc * 128 + 128],
                            thT[:, hc, :], start=(hc == 0), stop=False,
                            skip_group_check=True)
                    sc1 = fsb.tile([128, NE], BF, tag="fexp")
                    nc.scalar.activation(
                        sc1[:], fp[:], AF.Exp,
                        accum_out=sda[b][:, 2 * jc:2 * jc + 1])
                    for f in range(2):
                        nc.tensor.matmul(
                            fp[:], gbT[b][:, f, jc * 128:jc * 128 + 128],
                            tcT[b][:, f, :], start=False, stop=False,
                            skip_group_check=True)
                    nc.tensor.matmul(fp[:], negones[:, 0:128], lnZ[b][:],
                                     start=False, stop=True,
                                     skip_group_check=True)
                    sc2 = fsb.tile([128, NE], BF, tag="fexp")
                    nc.scalar.activation(
                        sc2[:], fp[:], AF.Exp,
                        accum_out=sda[b][:, 2 * jc + 1:2 * jc + 2])
            for b in range(BPC):
                lns = fsb.tile([128, 8], F32, tag="lns")
                nc.scalar.activation(lns[:], sda[b][:], AF.Ln)
                for jc in range(4):
                    nc.vector.tensor_sub(
                        pout_sb[:, b * 4 + jc:b * 4 + jc + 1],
                        lns[:, 2 * jc + 1:2 * jc + 2],
                        lns[:, 2 * jc:2 * jc + 1])
            nc.sync.dma_start(out=pout[:], in_=pout_sb[:])

    nc.compile()
    return nc


# ---------------------------------------------------------------------------
# host side
# ---------------------------------------------------------------------------

_CACHE = {}


def _get_program(n_enc, n_dec):
    key = (n_enc, n_dec)
    if key not in _CACHE:
        _CACHE[key] = build_program(n_enc, n_dec)
    return _CACHE[key]


def _host_prep(inputs, n_enc=NE, n_dec=ND):
    xs = np.asarray(inputs["xs_idx"]).astype(np.int64)
    ys = np.asarray(inputs["ys_idx"]).astype(np.int64)
    gembed_W = np.asarray(inputs["gembed_W"], np.float32)
    gconv_W = np.asarray(inputs["gconv_W"], np.float32)
    gdecode_W = np.asarray(inputs["gdecode_W"], np.float32)
    enc_embed = np.asarray(inputs["enc_embed"], np.float32)
    dec_embed = np.asarray(inputs["dec_embed"], np.float32)
    T = np.asarray(inputs["T"], np.float32)

    for nm in ("enc_b", "dec_b"):
        assert not np.any(np.asarray(inputs[nm])), f"{nm} nonzero unsupported"

    def lstm_w(wih, whh):
        wih = np.asarray(wih, np.float32)
        whh = np.asarray(whh, np.float32)
        wih_t = _bf(wih.T)
        whh_t = _bf(whh.T.reshape(2, 128, 4 * H).transpose(1, 0, 2))
        return wih_t, whh_t

    wih_d, whh_d = lstm_w(inputs["enc_Wih"], inputs["enc_Whh"])
    wiy_d, whd_d = lstm_w(inputs["dec_Wih"], inputs["dec_Whh"])

    w2_d = _bf(gdecode_W.reshape(2, 128, V).transpose(1, 0, 2))
    g = gconv_W.reshape(KW, 2, 128, 2, 128)
    gconv_d = _bf(np.ascontiguousarray(
        g.transpose(2, 0, 1, 3, 4).reshape(128, KW * 4, 128)))
    tt = T.T.reshape(4, 128, 2, 128)  # [ec, p, hc, c]
    tt_d = _bf(np.ascontiguousarray(
        tt.transpose(1, 0, 2, 3).reshape(128, 8, 128)))

    base = dict(
        gembed_bf=_bf(gembed_W), enc_embed_bf=_bf(enc_embed),
        dec_embed_bf=_bf(dec_embed), w2t_bf=_bf(gdecode_W.T),
        w2_d=w2_d, gconv_d=gconv_d, wih_d=wih_d, whh_d=whh_d,
        wiy_d=wiy_d, whd_d=whd_d, tt_d=tt_d,
    )

    in_maps = []
    for m in range(NCORES):
        order = np.concatenate(
            [np.arange(4 * m, 4 * m + 4),
             np.delete(np.arange(B), np.s_[4 * m:4 * m + 4])])
        xs_p, ys_p = xs[order], ys[order]
        xm = np.where(xs_p < PG, 0, xs_p)
        ym = np.where(ys_p < PG, 0, ys_p)
        im = dict(base)
        im["x_enc_idx"] = _wrap16(xm[:, :n_enc].T)   # (t,b) order
        im["y_dec_idx"] = _wrap16(ym[:, :n_dec].T)
        im["e_idx"] = _wrap16(xs_p[:BPC])            # (b,t) order
        im["gb_idx"] = _wrap16(ys_p[:BPC])
        in_maps.append(im)
    return in_maps


def kernel(**inputs):
    trace = bool(int(os.environ.get("KERNEL_TRACE", "0")))
    n_enc = int(os.environ.get("KERNEL_NENC", NE))
    n_dec = int(os.environ.get("KERNEL_NDEC", ND))
    nc = _get_program(n_enc, n_dec)
    in_maps = _host_prep(inputs, n_enc, n_dec)
    res = run_bass_kernel_spmd(nc, in_maps, list(range(NCORES)), trace=trace)
    total = np.float64(0.0)
    for r in res.results:
        total += np.asarray(r["pout"], np.float64).sum()
    kernel.last_results = res
    return np.float32(-total)



# revision 4
# speedup vs baseline: 1.0522x; 1.0522x over previous
"""Trainium2 Bass kernel for nn_EquivariantHardAlignmentModel — W4 design.

8 NeuronCores, pure data-parallel (B=4 rows per core, no replication):
  - LSTM recurrences in weight-stationary gate-tile layout: z lives as
    (128 gate-partitions, 8 gate-tiles, 4 batch) in PSUM.  The x@Wih part
    ("zx") is pre-accumulated into the PSUM blocks (16 steps per bank) by
    batched matmuls; per step only the 16 h@Whh tile-matmuls + 3 ACT +
    3 DVE ops run.  No transposes anywhere: h is produced directly in
    (128 h-partitions, 2, 4) layout and written straight into the henc/hdec
    stores that the final phase reads.
  - G-stack (embed/conv/logits/Z), ys gathers, bilinear alignment and loss
    tail identical to the streaming baseline (already per-core B=4).
  - p[b,j] = log(sum_i exp(lys+eij-lnZ)) - log(sum_i exp(eij)).  Host sums.
"""

import os
import sys

sys.path.insert(0, "/opt/trn_rl_repo")

import numpy as np
import ml_dtypes

import concourse.bass as bass
import concourse.mybir as mybir
import concourse.tile as tile
from concourse import bacc
from concourse.bass_utils import run_bass_kernel_spmd
from concourse.masks import make_identity

BF = mybir.dt.bfloat16
F32 = mybir.dt.float32
FP8 = mybir.dt.float8e4
AF = mybir.ActivationFunctionType
DR = mybir.MatmulPerfMode.DoubleRow

B, NE, ND = 32, 512, 512
V = 2000
H, F, KW, PG = 256, 256, 5, 4
EE, ED = 128, 128
NCORES, BPC = 8, 4
TBLK = 16  # LSTM steps per PSUM zx block (1 bank)

# torch gate blocks i,f,g,o (256 each) -> reordered [i f o g]
GATE_PERM = np.concatenate(
    [np.arange(0, 512), np.arange(768, 1024), np.arange(512, 768)]
)


def _bf(x):
    return np.ascontiguousarray(x.astype(ml_dtypes.bfloat16))


def _wrap16(flat):
    """index list -> (128, n/16) int16, dma_gather wrapped + 8x replicated."""
    flat = np.asarray(flat).reshape(-1)
    assert flat.size % 16 == 0
    w = flat.reshape(-1, 16).T.astype(np.int16)  # (16, n/16)
    return np.ascontiguousarray(np.tile(w, (8, 1)))


# ---------------------------------------------------------------------------
# device program
# ---------------------------------------------------------------------------

def build_program(n_enc=NE, n_dec=ND):
    from contextlib import ExitStack

    nc = bacc.Bacc(None, target_bir_lowering=False, debug=False)

    with tile.TileContext(nc) as tc, ExitStack() as es:
        dram = es.enter_context(tc.tile_pool(name="dram", bufs=1, space="DRAM"))

        def din(name, shape, dtype):
            return dram.tile(shape, dtype, kind="ExternalInput", name=name,
                             uniquify=False)

        x_enc_idx = din("x_enc_idx", [128, BPC * n_enc // 16], mybir.dt.int16)
        y_dec_idx = din("y_dec_idx", [128, BPC * n_dec // 16], mybir.dt.int16)
        e_idx = din("e_idx", [128, BPC * NE // 16], mybir.dt.int16)
        gb_idx = din("gb_idx", [128, BPC * ND // 16], mybir.dt.int16)
        gembed_bf = din("gembed_bf", [V, F], BF)
        enc_embed_bf = din("enc_embed_bf", [V, EE], BF)
        dec_embed_bf = din("dec_embed_bf", [V, ED], BF)
        w2t_bf = din("w2t_bf", [V, F], BF)
        w2_d = din("w2_d", [128, 2, V], BF)
        gconv_d = din("gconv_d", [128, KW * 4, 128], BF)
        wihT_d = din("wihT_d", [128, 8, 128], BF)
        whhT_d = din("whhT_d", [128, 2, 8, 128], FP8)
        wiyT_d = din("wiyT_d", [128, 8, 128], BF)
        whdT_d = din("whdT_d", [128, 2, 8, 128], FP8)
        tt_d = din("tt_d", [128, 8, 128], BF)
        pout = dram.tile([128, 16], F32, kind="ExternalOutput", name="pout",
                         uniquify=False)

        cpool = es.enter_context(tc.tile_pool(name="const", bufs=1))

        idbf = cpool.tile([128, 128], BF)
        make_identity(nc, idbf[:])
        idf32 = cpool.tile([128, 128], F32)
        make_identity(nc, idf32[:])
        negones = cpool.tile([1, 128], F32)
        nc.gpsimd.memset(negones[:], -1.0)

        def to_sbuf(ap, name):
            t = cpool.tile(list(ap.shape), ap.dtype, name=name)
            nc.sync.dma_start(out=t[:], in_=ap[:])
            return t

        w2_sb = to_sbuf(w2_d, "w2_sb")
        gconv_sb = to_sbuf(gconv_d, "gconv_sb")
        wihT = to_sbuf(wihT_d, "wihT")
        whhT = to_sbuf(whhT_d, "whhT")
        wiyT = to_sbuf(wiyT_d, "wiyT")
        whdT = to_sbuf(whdT_d, "whdT")
        tt_sb = to_sbuf(tt_d, "tt_sb")
        xidx_sb = to_sbuf(x_enc_idx, "xidx_sb")
        yidx_sb = to_sbuf(y_dec_idx, "yidx_sb")
        eidx_sb = to_sbuf(e_idx, "eidx_sb")
        gbidx_sb = to_sbuf(gb_idx, "gbidx_sb")

        gpool = es.enter_context(tc.tile_pool(name="gath", bufs=1))

        xgath = gpool.tile([128, 1, BPC * n_enc], BF)
        nc.gpsimd.dma_gather(
            out_ap=xgath[:], in_ap=enc_embed_bf[:], idxs_ap=xidx_sb[:],
            num_idxs=BPC * n_enc, num_idxs_reg=BPC * n_enc, elem_size=EE,
            transpose=True, single_packet=False)
        ygath = gpool.tile([128, 1, BPC * n_dec], BF)
        nc.gpsimd.dma_gather(
            out_ap=ygath[:], in_ap=dec_embed_bf[:], idxs_ap=yidx_sb[:],
            num_idxs=BPC * n_dec, num_idxs_reg=BPC * n_dec, elem_size=ED,
            transpose=True, single_packet=False)

        eT = [gpool.tile([128, 2, NE], BF, name=f"eT{b}") for b in range(BPC)]
        gbT = [gpool.tile([128, 2, ND], BF, name=f"gbT{b}") for b in range(BPC)]

        # persistent activation stores (b-major: col = b*T + t)
        spool = es.enter_context(tc.tile_pool(name="stores", bufs=1))
        tcT = [spool.tile([128, 2, NE], BF, name=f"tcT{b}") for b in range(BPC)]
        lnZ = [spool.tile([1, NE], F32, name=f"lnZ{b}") for b in range(BPC)]
        hencTf = spool.tile([128, 2, BPC * NE], FP8)
        hencTb = spool.tile([128, 2, BPC * NE], FP8)
        hdecT = spool.tile([128, 2, BPC * (ND + 1)], FP8)
        hencTf_bf = spool.tile([128, 2, BPC * NE], BF)
        hencTb_bf = spool.tile([128, 2, BPC * NE], BF)
        hdecT_bf = spool.tile([128, 2, BPC * (ND + 1)], BF)
        pout_sb = spool.tile([128, 16], F32)

        # ------------------------------------------------------------------
        # Phase G (identical to baseline)
        # ------------------------------------------------------------------
        with tc.tile_pool(name="gwork", bufs=2) as gw, \
             tc.tile_pool(name="gpsum", bufs=2, space="PSUM") as gp, \
             tc.tile_pool(name="zrow", bufs=4, space="PSUM") as zrp:
            for b in range(BPC):
                gsc = gw.tile([128, 2, NE], BF, tag="gathsc")
                nc.gpsimd.dma_gather(
                    out_ap=gsc[:], in_ap=gembed_bf[:],
                    idxs_ap=eidx_sb[:, b * NE // 16:(b + 1) * NE // 16],
                    num_idxs=NE, num_idxs_reg=NE, elem_size=F, transpose=True)
                nc.scalar.activation(eT[b][:], gsc[:], AF.Tanh)
                nc.gpsimd.dma_gather(
                    out_ap=gbT[b][:], in_ap=w2t_bf[:],
                    idxs_ap=gbidx_sb[:, b * ND // 16:(b + 1) * ND // 16],
                    num_idxs=ND, num_idxs_reg=ND, elem_size=F, transpose=True)
            # conv + tanh
            for b in range(BPC):
                for fo in range(2):
                    cp = gp.tile([128, NE], F32, tag="convps")
                    first = True
                    for k in [2, 0, 1, 3, 4]:
                        d = k - 2
                        lo_out, lo_in = max(0, -d), max(0, d)
                        L = NE - abs(d)
                        for fi in range(2):
                            nc.tensor.matmul(
                                cp[:, lo_out:lo_out + L],
                                gconv_sb[:, (k * 2 + fi) * 2 + fo, :],
                                eT[b][:, fi, lo_in:lo_in + L],
                                start=first, stop=(k == 4 and fi == 1),
                                skip_group_check=True)
                            first = False
                    nc.scalar.activation(tcT[b][:, fo, :], cp[:], AF.Tanh)
            # logits (t-major) -> exp -> Z
            zrows = []
            for b in range(BPC):
                zrow = zrp.tile([1, NE], F32, tag="zrow", name=f"zr{b}")
                for ic in range(4):
                    zp = gw.tile([128, 4], F32, tag="zp")
                    for vc in range(4):
                        lp = gp.tile([128, 500], F32, tag="logps")
                        for f in range(2):
                            nc.tensor.matmul(
                                lp[:], tcT[b][:, f, ic * 128:(ic + 1) * 128],
                                w2_sb[:, f, vc * 500:(vc + 1) * 500],
                                start=(f == 0), stop=(f == 1))
                        sc = gw.tile([128, 500], BF, tag="expsc")
                        nc.scalar.activation(sc[:], lp[:], AF.Exp,
                                             accum_out=zp[:, vc:vc + 1])
                    zc = gw.tile([128, 1], F32, tag="zc")
                    nc.vector.tensor_reduce(zc[:], zp[:],
                                            axis=mybir.AxisListType.X,
                                            op=mybir.AluOpType.add)
                    nc.tensor.transpose(zrow[:, ic * 128:(ic + 1) * 128],
                                        zc[:], idf32[:])
                zrows.append(zrow)
            for b in range(BPC):
                nc.scalar.activation(lnZ[b][:], zrows[b][:], AF.Ln)

        # ------------------------------------------------------------------
        # LSTM phases (weight-stationary, gate-tile layout)
        # ------------------------------------------------------------------
        lstm_es = ExitStack()
        lst_sb = lstm_es.enter_context(tc.tile_pool(name="lstm_sb", bufs=4))
        lst_st = lstm_es.enter_context(tc.tile_pool(name="lstm_st", bufs=1))
        zx_ps = lstm_es.enter_context(tc.tile_pool(name="zx_ps", bufs=2,
                                                   space="PSUM"))

        class WChain:
            """One B=4 recurrence.  store col = b*stride + col_of(t)."""

            def __init__(self, name, whT, wiT, xg, store, stride, col_of,
                         xpos, n_steps):
                self.name = name
                self.whT, self.wiT, self.xg = whT, wiT, xg
                self.store, self.stride = store, stride
                self.col_of, self.xpos = col_of, xpos
                self.n = n_steps
                self.GC = None   # (128, 4, 4) bf16 [g0 g1 c0 c1]
                self.H0 = None   # (128, 2, 4) initial h
                self.blk = None
                self.nblk = None

            def init_zero(self):
                self.GC = lst_st.tile([128, 4, 4], BF, name=self.name + "GC")
                nc.gpsimd.memset(self.GC[:], 0.0)
                self.H0 = lst_st.tile([128, 2, 4], FP8, name=self.name + "H0")
                nc.gpsimd.memset(self.H0[:], 0.0)

            def fill_block(self, t0):
                """zx for steps [t0, t0+TBLK): block slot s holds enc/dec
                position pmin+s where pmin = min over the window of xpos."""
                blk = zx_ps.tile([128, 8, TBLK, 4], F32, tag=self.name + "zx")
                ps = [self.xpos(t) for t in range(t0, t0 + TBLK)]
                pmin = min(ps)
                for gt in range(8):
                    nc.tensor.matmul(
                        blk[:, gt, :, :], self.wiT[:, gt, :],
                        self.xg[:, 0, pmin * 4:(pmin + TBLK) * 4],
                        start=True, stop=False, skip_group_check=True)
                return blk, pmin

            def h_rhs(self, t):
                if t == 0:
                    return self.H0
                c = self.col_of(t - 1)
                return self.store[:, :, c::self.stride][:, :, 0:BPC]

            def _mm(self, z, rhs, gt):
                # fp8 DoubleRow: both k-tiles contracted in one matmul
                nc.tensor.matmul(
                    z[:, gt, :], self.whT[:, :, gt, :], rhs,
                    start=False, stop=True, perf_mode=DR,
                    skip_group_check=True)

            def s1(self, t):
                if t % TBLK == 0:
                    self.blk = self.fill_block(0) if t == 0 else self.nblk
                blk, pmin = self.blk
                t_in = self.xpos(t) - pmin
                z = blk[:, :, t_in, :]           # (128, 8, 4)
                rhs = self.h_rhs(t)
                S = lst_sb.tile([128, 6, 4], BF, tag="S")
                # g first so tanh(g) starts while i,f,o stream
                for gt in (6, 7):
                    self._mm(z, rhs, gt)
                nc.scalar.activation(self.GC[:, 0:2, :], z[:, 6:8, :],
                                     AF.Tanh)
                for gt in range(6):
                    self._mm(z, rhs, gt)
                nc.scalar.activation(S[:], z[:, 0:6, :], AF.Sigmoid)
                self._S = S

            def s2(self, t):
                S = self._S
                M = lst_sb.tile([128, 4, 4], BF, tag="M")
                nc.vector.tensor_mul(M[:], S[:, 0:4, :], self.GC[:])
                nc.vector.tensor_add(self.GC[:, 2:4, :], M[:, 0:2, :],
                                     M[:, 2:4, :])
                T = lst_sb.tile([128, 2, 4], BF, tag="T")
                nc.scalar.activation(T[:], self.GC[:, 2:4, :], AF.Tanh)
                c = self.col_of(t)
                out = self.store[:, :, c::self.stride][:, :, 0:BPC]
                nc.vector.tensor_mul(out, S[:, 4:6, :], T[:])
                # prefetch next zx block off the critical path
                if t % TBLK == 0 and t + TBLK < self.n:
                    self.nblk = self.fill_block(t + TBLK)

        fwd = WChain("f", whhT, wihT, xgath, hencTf, NE, lambda t: t,
                     lambda t: t, n_enc)
        bwd = WChain("b", whhT, wihT, xgath, hencTb, NE,
                     lambda t: n_enc - 1 - t, lambda t: n_enc - 1 - t, n_enc)
        fwd.init_zero()
        bwd.init_zero()

        # 2:1 interleave: bwd runs at half rate so its second half can keep
        # the dec phase two-chained
        bt = 0
        for t in range(n_enc):
            fwd.s1(t)
            do_b = (t % 2 == 1 and bt < n_enc // 2)
            if do_b:
                bwd.s1(bt)
            fwd.s2(t)
            if do_b:
                bwd.s2(bt)
                bt += 1

        # dec: init from fwd final state; h(t-1) read from hdecT col t
        hTf = hencTf[:, :, (n_enc - 1)::NE][:, :, 0:BPC]
        nc.vector.tensor_copy(hdecT[:, :, 0::(ND + 1)][:, :, 0:BPC], hTf)

        dec = WChain("d", whdT, wiyT, ygath, hdecT, ND + 1,
                     lambda t: t + 1, lambda t: t, n_dec)
        dec.GC = fwd.GC
        dec.H0 = None  # unused: col_of(-1)=0 -> hdecT col 0 = hTf

        def dec_h_rhs(t):
            c = dec.col_of(t - 1) if t > 0 else 0
            return hdecT[:, :, c::(ND + 1)][:, :, 0:BPC]
        dec.h_rhs = dec_h_rhs

        for t in range(n_dec):
            dec.s1(t)
            do_b = (t % 2 == 1 and bt < n_enc)
            if do_b:
                bwd.s1(bt)
            dec.s2(t)
            if do_b:
                bwd.s2(bt)
                bt += 1
        while bt < n_enc:
            bwd.s1(bt)
            bwd.s2(bt)
            bt += 1

        lstm_es.close()

        # bulk-cast fp8 h stores to bf16 for the final-phase matmuls
        nc.vector.tensor_copy(hencTf_bf[:], hencTf[:])
        nc.vector.tensor_copy(hencTb_bf[:], hencTb[:])
        nc.vector.tensor_copy(hdecT_bf[:], hdecT[:])

        # ------------------------------------------------------------------
        # Final phase (identical to baseline)
        # ------------------------------------------------------------------
        with tc.tile_pool(name="fin_sb", bufs=2) as fsb, \
             tc.tile_pool(name="fin_keep", bufs=1) as fkeep, \
             tc.tile_pool(name="fin_ps", bufs=2, space="PSUM") as fps:
            sda = [fkeep.tile([128, 8], F32, name=f"sda{b}")
                   for b in range(BPC)]
            for b in range(BPC):
                thT = fsb.tile([128, 2, NE], BF, tag="thT")
                for hc in range(2):
                    tp = fps.tile([128, NE], F32, tag="thps")
                    for ec in range(4):
                        src = hencTf_bf if ec < 2 else hencTb_bf
                        nc.tensor.matmul(
                            tp[:], tt_sb[:, ec * 2 + hc, :],
                            src[:, ec % 2, b * NE:(b + 1) * NE],
                            start=(ec == 0), stop=(ec == 3))
                    nc.scalar.activation(thT[:, hc, :], tp[:], AF.Copy)
                for jc in range(4):
                    fp = fps.tile([128, NE], F32, tag="fps")
                    for hc in range(2):
                        nc.tensor.matmul(
                            fp[:],
                            hdecT_bf[:, hc, :][:, b * (ND + 1) + jc * 128:
                                               b * (ND + 1) + jc * 128 + 128],
                            thT[:, hc, :], start=(hc == 0), stop=False,
                            skip_group_check=True)
                    sc1 = fsb.tile([128, NE], BF, tag="fexp")
                    nc.scalar.activation(
                        sc1[:], fp[:], AF.Exp,
                        accum_out=sda[b][:, 2 * jc:2 * jc + 1])
                    for f in range(2):
                        nc.tensor.matmul(
                            fp[:], gbT[b][:, f, jc * 128:jc * 128 + 128],
                            tcT[b][:, f, :], start=False, stop=False,
                            skip_group_check=True)
                    nc.tensor.matmul(fp[:], negones[:, 0:128], lnZ[b][:],
                                     start=False, stop=True,
                                     skip_group_check=True)
                    sc2 = fsb.tile([128, NE], BF, tag="fexp")
                    nc.scalar.activation(
                        sc2[:], fp[:], AF.Exp,
                        accum_out=sda[b][:, 2 * jc + 1:2 * jc + 2])
            for b in range(BPC):
                lns = fsb.tile([128, 8], F32, tag="lns")
                nc.scalar.activation(lns[:], sda[b][:], AF.Ln)
                for jc in range(4):
                    nc.vector.tensor_sub(
                        pout_sb[:, b * 4 + jc:b * 4 + jc + 1],
                        lns[:, 2 * jc + 1:2 * jc + 2],
                        lns[:, 2 * jc:2 * jc + 1])
            nc.sync.dma_start(out=pout[:], in_=pout_sb[:])

    nc.compile()
    return nc


# ---------------------------------------------------------------------------
# host side
# ---------------------------------------------------------------------------

_CACHE = {}


def _get_program(n_enc, n_dec):
    key = (n_enc, n_dec)
    if key not in _CACHE:
        _CACHE[key] = build_program(n_enc, n_dec)
    return _CACHE[key]


def _host_prep(inputs, n_enc=NE, n_dec=ND):
    xs = np.asarray(inputs["xs_idx"]).astype(np.int64)
    ys = np.asarray(inputs["ys_idx"]).astype(np.int64)
    gembed_W = np.asarray(inputs["gembed_W"], np.float32)
    gconv_W = np.asarray(inputs["gconv_W"], np.float32)
    gdecode_W = np.asarray(inputs["gdecode_W"], np.float32)
    enc_embed = np.asarray(inputs["enc_embed"], np.float32)
    dec_embed = np.asarray(inputs["dec_embed"], np.float32)
    T = np.asarray(inputs["T"], np.float32)

    for nm in ("enc_b", "dec_b"):
        assert not np.any(np.asarray(inputs[nm])), f"{nm} nonzero unsupported"

    def lstm_w(wih, whh):
        wih = np.asarray(wih, np.float32)[GATE_PERM]   # (1024, E)
        whh = np.asarray(whh, np.float32)[GATE_PERM]   # (1024, H)
        wihT = wih.T.reshape(128, 8, 128)              # [e][gt][g]
        whhT = whh.T.reshape(2, 128, 8, 128).transpose(1, 0, 2, 3)
        whhT8 = np.ascontiguousarray(whhT).astype(ml_dtypes.float8_e4m3)
        return _bf(wihT), whhT8

    wihT_d, whhT_d = lstm_w(inputs["enc_Wih"], inputs["enc_Whh"])
    wiyT_d, whdT_d = lstm_w(inputs["dec_Wih"], inputs["dec_Whh"])

    w2_d = _bf(gdecode_W.reshape(2, 128, V).transpose(1, 0, 2))
    g = gconv_W.reshape(KW, 2, 128, 2, 128)
    gconv_d = _bf(np.ascontiguousarray(
        g.transpose(2, 0, 1, 3, 4).reshape(128, KW * 4, 128)))
    tt = T.T.reshape(4, 128, 2, 128)  # [ec, p, hc, c]
    tt_d = _bf(np.ascontiguousarray(
        tt.transpose(1, 0, 2, 3).reshape(128, 8, 128)))

    base = dict(
        gembed_bf=_bf(gembed_W), enc_embed_bf=_bf(enc_embed),
        dec_embed_bf=_bf(dec_embed), w2t_bf=_bf(gdecode_W.T),
        w2_d=w2_d, gconv_d=gconv_d, wihT_d=wihT_d, whhT_d=whhT_d,
        wiyT_d=wiyT_d, whdT_d=whdT_d, tt_d=tt_d,
    )

    in_maps = []
    for m in range(NCORES):
        rows = np.arange(4 * m, 4 * m + 4)
        xs_p, ys_p = xs[rows], ys[rows]
        xm = np.where(xs_p < PG, 0, xs_p)
        ym = np.where(ys_p < PG, 0, ys_p)
        im = dict(base)
        im["x_enc_idx"] = _wrap16(xm[:, :n_enc].T)   # (t,b) order
        im["y_dec_idx"] = _wrap16(ym[:, :n_dec].T)
        im["e_idx"] = _wrap16(xs_p)                  # (b,t) order
        im["gb_idx"] = _wrap16(ys_p)
        in_maps.append(im)
    return in_maps


def kernel(**inputs):
    trace = bool(int(os.environ.get("KERNEL_TRACE", "0")))
    n_enc = int(os.environ.get("KERNEL_NENC", NE))
    n_dec = int(os.environ.get("KERNEL_NDEC", ND))
    nc = _get_program(n_enc, n_dec)
    in_maps = _host_prep(inputs, n_enc, n_dec)
    res = run_bass_kernel_spmd(nc, in_maps, list(range(NCORES)), trace=trace)
    total = np.float64(0.0)
    for r in res.results:
        total += np.asarray(r["pout"], np.float64).sum()
    kernel.last_results = res
    return np.float32(-total)
